# revision 9
# baseline (speedup 1.0000x reference)
"""Trainium2 Bass kernel for the sparse-attention nn.Module.

Math (per batch b, all fp32):
    Q = Wq @ x1 + bq            [32, N]     (N = 128*128 = 16384)
    K = Wk @ x1 + bk            [32, N]
    V = Wv @ x  + bv            [192, N]
    Qn = Q / ||Q||_col, Kn = K / ||K||_col          (norm over channel dim)
    ksum[m]   = sum_n Kn[m, n]
    tailor[n] = 1 / (N + sum_m Qn[m, n] * (ksum[m] + EPS))
    vsum[c]   = sum_n V[c, n]
    matrix[m, c] = sum_n Kn[m, n] V[c, n]
    out[c, n] = gamma * tailor[n] * (vsum[c] + sum_m Qn[m, n] matrix[m, c])

Distribution: data-parallel over batch. B == 8 == n_cores; each core gets one
batch slice, no collectives. Host reshapes/stacks.

Device algorithm (pos-major, one pass over N + one small pass):
  phase 1 per 128-position sub-chunk s (128 of them):
    qk_ps[128, 64]  = [x1 | 1s]^T-slice matmuls against [WqkT; bqk]  (bias via
                      ones-row fold), pos-major: partition = position.
    v_ps[128, 192]  = same for V.
    per-position norms of Q and K via ACT Square(accum) + Sqrt + DVE recip.
    Qn stored to qbuf[128, 32*128]; kn_aug[128, 33] = [Kn | 1]; vt[128, 193] =
    [V^T | 1].
    mt_ps[33, 193] += kn_aug^T @ vt   -- one accumulating matmul: rows 0..31 =
                      matrix[m, c], row 32 = vsum[c], col 192 = ksum[m].
  phase 1.5: ksum row extraction (tiny PE transpose), broadcast of
    (ksum + EPS) via Kc=1 matmul, tailor per position, fold gamma into tailor,
    build Qnt_aug[33, N] = transpose of [Qn * tailor_g | tailor_g] per chunk.
  phase 2 per 512-position chunk: out[c, n] = mt_sb[:, c]^T @ Qnt_aug[:, n]
    (Kc = 33), direct [c, n] layout, DMA out. Optionally fp32r (1 cyc/row at
    free-dim 512 vs 4 for fp32).
"""

import numpy as np

import concourse.bass as bass
import concourse.mybir as mybir
import concourse.tile as tile
from concourse import bacc
from concourse.bass_utils import run_bass_kernel_spmd
from concourse.masks import make_identity

F32 = mybir.dt.float32
AX = mybir.AxisListType
AF = mybir.ActivationFunctionType

N_CORES = 8
B, C, H, W = 8, 192, 128, 128
CQ = 32
N = H * W          # 16384
EPS = 1e-6

CHUNK = 1024       # positions per DMA chunk
NCHUNK = N // CHUNK            # 32
SUB = 128          # positions per matmul sub-chunk
NSUB = N // SUB                # 128
SUBS_PER_CHUNK = CHUNK // SUB
GRP = 16           # sub-chunks per phase-1.5 group
NGRP = NSUB // GRP             # 8

# fp32r runs the PE at 1 cycle/row (vs 4 for fp32, which also needs 2 HW
# passes) when the matmul free dim is >= 256 -- so pad rhs free dims to 256.
FP32R_PHASE1 = True
FP32R_PHASE2 = True
PADF = 256
F32R = mybir.dt.float32r
DT1 = F32R if FP32R_PHASE1 else F32   # phase-1 matmul operand storage dtype
DT2 = F32R if FP32R_PHASE2 else F32   # phase-2 matmul operand storage dtype


def _mm(nc, out, lhsT, rhs, fp32r, **kw):
    nc.tensor.matmul(out, lhsT=lhsT, rhs=rhs, **kw)


def build_program():
    nc = bacc.Bacc("TRN2", target_bir_lowering=False, debug=False,
                   num_devices=N_CORES)

    x1 = nc.dram_tensor("x1", [C, N], DT1, kind="ExternalInput").ap()
    x0 = nc.dram_tensor("x0", [C, N], DT1, kind="ExternalInput").ap()
    wqk1 = nc.dram_tensor("wqk1", [128, PADF], DT1, kind="ExternalInput").ap()
    wqk2 = nc.dram_tensor("wqk2", [65, PADF], DT1, kind="ExternalInput").ap()
    wv1 = nc.dram_tensor("wv1", [128, PADF], DT1, kind="ExternalInput").ap()
    wv2 = nc.dram_tensor("wv2", [65, PADF], DT1, kind="ExternalInput").ap()
    gam = nc.dram_tensor("gam", [1, 1], F32, kind="ExternalInput").ap()
    ones_d = nc.dram_tensor("ones_d", [1, CHUNK], DT1, kind="ExternalInput").ap()
    out = nc.dram_tensor("out", [C, N], F32, kind="ExternalOutput").ap()

    with tile.TileContext(nc) as tc:
        with tc.tile_pool(name="singles", bufs=1) as singles, \
             tc.tile_pool(name="mtps", bufs=1, space="PSUM") as mtps_pool:
            w_qk1 = singles.tile([128, PADF], DT1)
            nc.sync.dma_start(out=w_qk1, in_=wqk1)
            w_qk2 = singles.tile([65, PADF], DT1)
            nc.sync.dma_start(out=w_qk2, in_=wqk2)
            w_v1 = singles.tile([128, PADF], DT1)
            nc.sync.dma_start(out=w_v1, in_=wv1)
            w_v2 = singles.tile([65, PADF], DT1)
            nc.sync.dma_start(out=w_v2, in_=wv2)
            gamma_bc = singles.tile([128, 1], F32)
            nc.sync.dma_start(out=gamma_bc, in_=gam.to_broadcast([128, 1]))
            ident = singles.tile([128, 128], F32)
            make_identity(nc, ident)
            ones_row = singles.tile([1, 128], F32)
            nc.vector.memset(ones_row, 1.0)

            # manually-rotated persistent tiles: the ones-regions are written
            # once here and never touched by the per-iteration writes, which
            # only cover the data region (avoids per-iteration memsets, which
            # cannot encode float32r anyway).
            x1b_t = [singles.tile([65, CHUNK], DT1, tag=f"x1b{i}", name=f"x1b{i}") for i in range(2)]
            xb_t = [singles.tile([65, CHUNK], DT1, tag=f"xb{i}", name=f"xb{i}") for i in range(2)]
            for t in x1b_t + xb_t:
                nc.sync.dma_start(out=t[64:65], in_=ones_d)
            kn_t = [singles.tile([128, CQ + 1], DT1, tag=f"kn{i}", name=f"kn{i}") for i in range(3)]
            for t in kn_t:
                nc.sync.dma_start(
                    out=t[:, CQ:CQ + 1],
                    in_=ones_d[0:1, 0:1].to_broadcast([128, 1]))
            vt_t = [singles.tile([128, PADF], DT1, tag=f"vt{i}", name=f"vt{i}") for i in range(3)]
            for t in vt_t:
                nc.sync.dma_start(
                    out=t[:, C:PADF],
                    in_=ones_d[0:1, 0:PADF - C].to_broadcast([128, PADF - C]))

            qbuf = singles.tile([128, NSUB * CQ], F32)     # Qn, pos-major
            qtaug = singles.tile([CQ + 1, N], DT2)         # Qnt_aug, chan-major
            mt_sb = singles.tile([CQ + 1, C + 1], DT2)     # matrix_aug
            kse_rep = singles.tile([128, GRP * CQ], F32)   # (ksum+EPS) bcast

            mt_ps = mtps_pool.tile([CQ + 1, PADF], F32)

            # ---------------- phase 1 ----------------
            with tc.tile_pool(name="xin", bufs=2) as xin, \
                 tc.tile_pool(name="qkps", bufs=2, space="PSUM") as qkps, \
                 tc.tile_pool(name="vps", bufs=2, space="PSUM") as vps, \
                 tc.tile_pool(name="p1sm", bufs=3) as p1sm:
                for ci in range(NCHUNK):
                    n0 = ci * CHUNK
                    x1a = xin.tile([128, CHUNK], DT1, tag="x1a")
                    nc.sync.dma_start(out=x1a, in_=x1[0:128, n0:n0 + CHUNK])
                    x1b = x1b_t[ci % 2]
                    nc.sync.dma_start(out=x1b[0:64], in_=x1[128:C, n0:n0 + CHUNK])
                    xa = xin.tile([128, CHUNK], DT1, tag="xa")
                    nc.sync.dma_start(out=xa, in_=x0[0:128, n0:n0 + CHUNK])
                    xb = xb_t[ci % 2]
                    nc.sync.dma_start(out=xb[0:64], in_=x0[128:C, n0:n0 + CHUNK])

                    for si in range(SUBS_PER_CHUNK):
                        sub = ci * SUBS_PER_CHUNK + si
                        sl = slice(si * SUB, (si + 1) * SUB)

                        qk_ps = qkps.tile([128, PADF], F32, tag="qk")
                        _mm(nc, qk_ps, x1a[:, sl], w_qk1, FP32R_PHASE1,
                            start=True, stop=False)
                        _mm(nc, qk_ps, x1b[:, sl], w_qk2, FP32R_PHASE1,
                            start=False, stop=True)
                        v_ps = vps.tile([128, PADF], F32, tag="v")
                        _mm(nc, v_ps, xa[:, sl], w_v1, FP32R_PHASE1,
                            start=True, stop=False)
                        _mm(nc, v_ps, xb[:, sl], w_v2, FP32R_PHASE1,
                            start=False, stop=True)

                        # per-position norms of Q and K (free-dim reduce)
                        scr = p1sm.tile([128, 2 * CQ], F32, tag="scr")
                        nc.scalar.activation(out=scr, in_=qk_ps[:, 0:2 * CQ],
                                             func=AF.Square)
                        sq2 = p1sm.tile([128, 2], F32, tag="sq2")
                        nc.vector.reduce_sum(
                            sq2, scr.rearrange("p (c k) -> p c k", k=CQ),
                            axis=AX.X)
                        rn2 = p1sm.tile([128, 2], F32, tag="rn2")
                        nc.scalar.sqrt(rn2, sq2)
                        nc.vector.reciprocal(rn2, rn2)

                        nc.vector.tensor_scalar_mul(
                            qbuf[:, sub * CQ:(sub + 1) * CQ],
                            qk_ps[:, 0:CQ], rn2[:, 0:1])
                        kn = kn_t[sub % 3]
                        nc.vector.tensor_scalar_mul(
                            kn[:, 0:CQ], qk_ps[:, CQ:2 * CQ], rn2[:, 1:2])
                        vt = vt_t[sub % 3]
                        nc.vector.tensor_copy(vt[:, 0:C], v_ps[:, 0:C])

                        _mm(nc, mt_ps, kn, vt, FP32R_PHASE1,
                            start=(sub == 0), stop=(sub == NSUB - 1))

            # ---------------- phase 1.5 ----------------
            nc.vector.tensor_copy(mt_sb, mt_ps[:, 0:C + 1])

            with tc.tile_pool(name="p15ps", bufs=2, space="PSUM") as p15ps, \
                 tc.tile_pool(name="p15sm", bufs=3) as p15sm, \
                 tc.tile_pool(name="trps", bufs=3, space="PSUM") as trps:
                # ksum (col 192 of mt_sb, rows 0..31) -> row, + EPS, bcast
                ks_ps = p15ps.tile([1, CQ], F32, tag="ksps")
                nc.tensor.transpose(ks_ps, mt_sb[0:CQ, C:C + 1].bitcast(F32),
                                    ident[0:CQ, 0:CQ])
                kse_row = p15sm.tile([1, CQ], F32, tag="kser")
                nc.vector.tensor_scalar_add(kse_row, ks_ps, EPS)
                kb_ps = p15ps.tile([128, CQ], F32, tag="kbps")
                nc.tensor.matmul(kb_ps, lhsT=ones_row, rhs=kse_row)
                for r in range(GRP):
                    nc.vector.tensor_copy(kse_rep[:, r * CQ:(r + 1) * CQ],
                                          kb_ps)

                for g in range(NGRP):
                    prod = p15sm.tile([128, GRP * CQ], F32, tag="prod")
                    nc.vector.tensor_mul(
                        prod, qbuf[:, g * GRP * CQ:(g + 1) * GRP * CQ],
                        kse_rep)
                    dot = p15sm.tile([128, GRP], F32, tag="dot")
                    nc.vector.reduce_sum(
                        dot, prod.rearrange("p (c k) -> p c k", k=CQ),
                        axis=AX.X)
                    tg = p15sm.tile([128, GRP], F32, tag="tg")
                    nc.vector.tensor_scalar_add(tg, dot, float(N))
                    nc.vector.reciprocal(tg, tg)
                    nc.vector.tensor_scalar_mul(tg, tg, gamma_bc[:, 0:1])

                    for k in range(GRP):
                        sub = g * GRP + k
                        tr_in = p15sm.tile([128, CQ + 1], F32, tag="trin")
                        nc.vector.tensor_scalar_mul(
                            tr_in[:, 0:CQ],
                            qbuf[:, sub * CQ:(sub + 1) * CQ], tg[:, k:k + 1])
                        nc.gpsimd.tensor_copy(tr_in[:, CQ:CQ + 1], tg[:, k:k + 1])
                        tr_ps = trps.tile([CQ + 1, 128], F32, tag="trps")
                        nc.tensor.transpose(tr_ps, tr_in, ident)
                        nc.vector.tensor_copy(
                            qtaug[:, sub * SUB:(sub + 1) * SUB], tr_ps)

            # ---------------- phase 2 ----------------
            P2C = 1024
            with tc.tile_pool(name="p2ps", bufs=3, space="PSUM") as p2ps, \
                 tc.tile_pool(name="p2sb", bufs=2) as p2sb:
                lhs0 = mt_sb[:, 0:128]
                lhs1 = mt_sb[:, 128:C]
                for ci in range(N // P2C):
                    n0 = ci * P2C
                    ob0 = p2sb.tile([128, P2C], F32, tag="ob0")
                    ob1 = p2sb.tile([64, P2C], F32, tag="ob1")
                    for h in range(P2C // 512):
                        hs = slice(h * 512, (h + 1) * 512)
                        rhs = qtaug[:, n0 + h * 512:n0 + (h + 1) * 512]
                        o0 = p2ps.tile([128, 512], F32, tag="o0")
                        _mm(nc, o0, lhs0, rhs, FP32R_PHASE2)
                        nc.vector.tensor_copy(ob0[:, hs], o0)
                        o1 = p2ps.tile([64, 512], F32, tag="o1")
                        _mm(nc, o1, lhs1, rhs, FP32R_PHASE2)
                        nc.vector.tensor_copy(ob1[:, hs], o1)
                    nc.sync.dma_start(out=out[0:128, n0:n0 + P2C], in_=ob0)
                    nc.sync.dma_start(out=out[128:C, n0:n0 + P2C], in_=ob1)

    nc.compile()
    return nc


_NC = None


def _get_program():
    global _NC
    if _NC is None:
        _NC = build_program()
    return _NC


def _padf(a):
    out = np.zeros((a.shape[0], PADF), np.float32)
    out[:, :a.shape[1]] = a
    return out


def _host_prep(Wq, bq, Wk, bk, Wv, bv):
    WqkT = np.ascontiguousarray(np.concatenate([Wq, Wk], axis=0).T)  # [192, 64]
    bqk = np.concatenate([bq, bk], axis=0)[None, :]                  # [1, 64]
    wqk1 = _padf(WqkT[:128])
    wqk2 = _padf(np.concatenate([WqkT[128:], bqk], axis=0))
    WvT = np.ascontiguousarray(Wv.T)                                 # [192, 192]
    wv1 = _padf(WvT[:128])
    wv2 = _padf(np.concatenate([WvT[128:], bv[None, :]], axis=0))
    return wqk1, wqk2, wv1, wv2


def kernel(x, x1, Wq, bq, Wk, bk, Wv, bv, gamma):
    x = np.asarray(x, dtype=np.float32)
    x1 = np.asarray(x1, dtype=np.float32)
    wqk1, wqk2, wv1, wv2 = _host_prep(
        np.asarray(Wq, np.float32), np.asarray(bq, np.float32),
        np.asarray(Wk, np.float32), np.asarray(bk, np.float32),
        np.asarray(Wv, np.float32), np.asarray(bv, np.float32))
    gam = np.asarray(gamma, np.float32).reshape(1, 1)

    nc = _get_program()
    ones_one = np.ones((1, CHUNK), np.float32)
    in_maps = []
    for b in range(B):
        in_maps.append({
            "x1": np.ascontiguousarray(x1[b].reshape(C, N)),
            "x0": np.ascontiguousarray(x[b].reshape(C, N)),
            "wqk1": wqk1, "wqk2": wqk2, "wv1": wv1, "wv2": wv2,
            "gam": gam, "ones_d": ones_one,
        })
    res = run_bass_kernel_spmd(nc, in_maps, list(range(N_CORES)))
    outs = [res.results[b]["out"].reshape(C, H, W) for b in range(B)]
    return np.stack(outs, axis=0)


# revision 10
# speedup vs baseline: 1.0297x; 1.0297x over previous
"""Trainium2 Bass kernel for the sparse-attention nn.Module.

Math (per batch b, all fp32):
    Q = Wq @ x1 + bq            [32, N]     (N = 128*128 = 16384)
    K = Wk @ x1 + bk            [32, N]
    V = Wv @ x  + bv            [192, N]
    Qn = Q / ||Q||_col, Kn = K / ||K||_col          (norm over channel dim)
    ksum[m]   = sum_n Kn[m, n]
    tailor[n] = 1 / (N + sum_m Qn[m, n] * (ksum[m] + EPS))
    vsum[c]   = sum_n V[c, n]
    matrix[m, c] = sum_n Kn[m, n] V[c, n]
    out[c, n] = gamma * tailor[n] * (vsum[c] + sum_m Qn[m, n] matrix[m, c])

Distribution: data-parallel over batch. B == 8 == n_cores; each core gets one
batch slice, no collectives. Host reshapes/stacks.

Device algorithm (pos-major, one pass over N + one small pass):
  phase 1 per 128-position sub-chunk s (128 of them):
    qk_ps[128, 64]  = [x1 | 1s]^T-slice matmuls against [WqkT; bqk]  (bias via
                      ones-row fold), pos-major: partition = position.
    v_ps[128, 192]  = same for V.
    per-position norms of Q and K via ACT Square(accum) + Sqrt + DVE recip.
    Qn stored to qbuf[128, 32*128]; kn_aug[128, 33] = [Kn | 1]; vt[128, 193] =
    [V^T | 1].
    mt_ps[33, 193] += kn_aug^T @ vt   -- one accumulating matmul: rows 0..31 =
                      matrix[m, c], row 32 = vsum[c], col 192 = ksum[m].
  phase 1.5: ksum row extraction (tiny PE transpose), broadcast of
    (ksum + EPS) via Kc=1 matmul, tailor per position, fold gamma into tailor,
    build Qnt_aug[33, N] = transpose of [Qn * tailor_g | tailor_g] per chunk.
  phase 2 per 512-position chunk: out[c, n] = mt_sb[:, c]^T @ Qnt_aug[:, n]
    (Kc = 33), direct [c, n] layout, DMA out. Optionally fp32r (1 cyc/row at
    free-dim 512 vs 4 for fp32).
"""

import numpy as np

import concourse.bass as bass
import concourse.mybir as mybir
import concourse.tile as tile
from concourse import bacc
from concourse.bass_utils import run_bass_kernel_spmd
from concourse.masks import make_identity

F32 = mybir.dt.float32
AX = mybir.AxisListType
AF = mybir.ActivationFunctionType

N_CORES = 8
B, C, H, W = 8, 192, 128, 128
CQ = 32
N = H * W          # 16384
EPS = 1e-6

CHUNK = 512        # positions per DMA chunk
NCHUNK = N // CHUNK            # 32
SUB = 128          # positions per matmul sub-chunk
NSUB = N // SUB                # 128
SUBS_PER_CHUNK = CHUNK // SUB
GRP = 16           # sub-chunks per phase-1.5 group
NGRP = NSUB // GRP             # 8

# fp32r runs the PE at 1 cycle/row (vs 4 for fp32, which also needs 2 HW
# passes) when the matmul free dim is >= 256 -- so pad rhs free dims to 256.
FP32R_PHASE1 = True
FP32R_PHASE2 = True
PADF = 256
F32R = mybir.dt.float32r
DT1 = F32R if FP32R_PHASE1 else F32   # phase-1 matmul operand storage dtype
DT2 = F32R if FP32R_PHASE2 else F32   # phase-2 matmul operand storage dtype


def _mm(nc, out, lhsT, rhs, fp32r, **kw):
    nc.tensor.matmul(out, lhsT=lhsT, rhs=rhs, **kw)


def build_program():
    nc = bacc.Bacc("TRN2", target_bir_lowering=False, debug=False,
                   num_devices=N_CORES)

    x1 = nc.dram_tensor("x1", [C, N], DT1, kind="ExternalInput").ap()
    x0 = nc.dram_tensor("x0", [C, N], DT1, kind="ExternalInput").ap()
    wqk1 = nc.dram_tensor("wqk1", [128, PADF], DT1, kind="ExternalInput").ap()
    wqk2 = nc.dram_tensor("wqk2", [65, PADF], DT1, kind="ExternalInput").ap()
    wv1 = nc.dram_tensor("wv1", [128, PADF], DT1, kind="ExternalInput").ap()
    wv2 = nc.dram_tensor("wv2", [65, PADF], DT1, kind="ExternalInput").ap()
    gam = nc.dram_tensor("gam", [1, 1], F32, kind="ExternalInput").ap()
    ones_d = nc.dram_tensor("ones_d", [1, CHUNK], DT1, kind="ExternalInput").ap()
    out = nc.dram_tensor("out", [C, N], F32, kind="ExternalOutput").ap()

    with tile.TileContext(nc) as tc:
        with tc.tile_pool(name="singles", bufs=1) as singles, \
             tc.tile_pool(name="mtps", bufs=1, space="PSUM") as mtps_pool:
            w_qk1 = singles.tile([128, PADF], DT1)
            nc.sync.dma_start(out=w_qk1, in_=wqk1)
            w_qk2 = singles.tile([65, PADF], DT1)
            nc.sync.dma_start(out=w_qk2, in_=wqk2)
            w_v1 = singles.tile([128, PADF], DT1)
            nc.sync.dma_start(out=w_v1, in_=wv1)
            w_v2 = singles.tile([65, PADF], DT1)
            nc.sync.dma_start(out=w_v2, in_=wv2)
            gamma_bc = singles.tile([128, 1], F32)
            nc.sync.dma_start(out=gamma_bc, in_=gam.to_broadcast([128, 1]))
            ident = singles.tile([128, 128], F32)
            make_identity(nc, ident)
            ones_row = singles.tile([1, 128], F32)
            nc.vector.memset(ones_row, 1.0)

            # manually-rotated persistent tiles: the ones-regions are written
            # once here and never touched by the per-iteration writes, which
            # only cover the data region (avoids per-iteration memsets, which
            # cannot encode float32r anyway).
            x1b_t = [singles.tile([65, CHUNK], DT1, tag=f"x1b{i}", name=f"x1b{i}") for i in range(2)]
            xb_t = [singles.tile([65, CHUNK], DT1, tag=f"xb{i}", name=f"xb{i}") for i in range(2)]
            for t in x1b_t + xb_t:
                nc.sync.dma_start(out=t[64:65], in_=ones_d)
            kn_t = [singles.tile([128, CQ + 1], DT1, tag=f"kn{i}", name=f"kn{i}") for i in range(3)]
            for t in kn_t:
                nc.sync.dma_start(
                    out=t[:, CQ:CQ + 1],
                    in_=ones_d[0:1, 0:1].to_broadcast([128, 1]))
            vt_t = [singles.tile([128, PADF], DT1, tag=f"vt{i}", name=f"vt{i}") for i in range(3)]
            for t in vt_t:
                nc.sync.dma_start(
                    out=t[:, C:PADF],
                    in_=ones_d[0:1, 0:PADF - C].to_broadcast([128, PADF - C]))

            qbuf = singles.tile([128, NSUB * CQ], F32)     # Qn, pos-major
            qtaug = singles.tile([CQ + 1, N], DT2)         # Qnt_aug, chan-major
            mt_sb = singles.tile([CQ + 1, C + 1], DT2)     # matrix_aug
            kse_rep = singles.tile([128, GRP * CQ], F32)   # (ksum+EPS) bcast

            mt_ps = mtps_pool.tile([CQ + 1, PADF], F32)

            # ---------------- phase 1 ----------------
            with tc.tile_pool(name="xin", bufs=2) as xin, \
                 tc.tile_pool(name="qkps", bufs=2, space="PSUM") as qkps, \
                 tc.tile_pool(name="vps", bufs=2, space="PSUM") as vps, \
                 tc.tile_pool(name="p1sm", bufs=3) as p1sm:
                for ci in range(NCHUNK):
                    n0 = ci * CHUNK
                    x1a = xin.tile([128, CHUNK], DT1, tag="x1a")
                    nc.gpsimd.dma_start(out=x1a, in_=x1[0:128, n0:n0 + CHUNK])
                    x1b = x1b_t[ci % 2]
                    nc.gpsimd.dma_start(out=x1b[0:64], in_=x1[128:C, n0:n0 + CHUNK])
                    xa = xin.tile([128, CHUNK], DT1, tag="xa")
                    nc.gpsimd.dma_start(out=xa, in_=x0[0:128, n0:n0 + CHUNK])
                    xb = xb_t[ci % 2]
                    nc.gpsimd.dma_start(out=xb[0:64], in_=x0[128:C, n0:n0 + CHUNK])

                    for si in range(SUBS_PER_CHUNK):
                        sub = ci * SUBS_PER_CHUNK + si
                        sl = slice(si * SUB, (si + 1) * SUB)

                        qk_ps = qkps.tile([128, PADF], F32, tag="qk")
                        _mm(nc, qk_ps, x1a[:, sl], w_qk1, FP32R_PHASE1,
                            start=True, stop=False)
                        _mm(nc, qk_ps, x1b[:, sl], w_qk2, FP32R_PHASE1,
                            start=False, stop=True)
                        v_ps = vps.tile([128, PADF], F32, tag="v")
                        _mm(nc, v_ps, xa[:, sl], w_v1, FP32R_PHASE1,
                            start=True, stop=False)
                        _mm(nc, v_ps, xb[:, sl], w_v2, FP32R_PHASE1,
                            start=False, stop=True)

                        # per-position norms of Q and K (free-dim reduce)
                        scr = p1sm.tile([128, 2 * CQ], F32, tag="scr")
                        nc.scalar.activation(out=scr, in_=qk_ps[:, 0:2 * CQ],
                                             func=AF.Square)
                        sq2 = p1sm.tile([128, 2], F32, tag="sq2")
                        nc.vector.reduce_sum(
                            sq2, scr.rearrange("p (c k) -> p c k", k=CQ),
                            axis=AX.X)
                        rn2 = p1sm.tile([128, 2], F32, tag="rn2")
                        nc.scalar.sqrt(rn2, sq2)
                        nc.vector.reciprocal(rn2, rn2)

                        nc.vector.tensor_scalar_mul(
                            qbuf[:, sub * CQ:(sub + 1) * CQ],
                            qk_ps[:, 0:CQ], rn2[:, 0:1])
                        kn = kn_t[sub % 3]
                        nc.vector.tensor_scalar_mul(
                            kn[:, 0:CQ], qk_ps[:, CQ:2 * CQ], rn2[:, 1:2])
                        vt = vt_t[sub % 3]
                        nc.vector.tensor_copy(vt[:, 0:C], v_ps[:, 0:C])

                        _mm(nc, mt_ps, kn, vt, FP32R_PHASE1,
                            start=(sub == 0), stop=(sub == NSUB - 1))

            # ---------------- phase 1.5 ----------------
            nc.vector.tensor_copy(mt_sb, mt_ps[:, 0:C + 1])

            with tc.tile_pool(name="p15ps", bufs=2, space="PSUM") as p15ps, \
                 tc.tile_pool(name="p15sm", bufs=3) as p15sm, \
                 tc.tile_pool(name="trps", bufs=3, space="PSUM") as trps:
                # ksum (col 192 of mt_sb, rows 0..31) -> row, + EPS, bcast
                ks_ps = p15ps.tile([1, CQ], F32, tag="ksps")
                nc.tensor.transpose(ks_ps, mt_sb[0:CQ, C:C + 1].bitcast(F32),
                                    ident[0:CQ, 0:CQ])
                kse_row = p15sm.tile([1, CQ], F32, tag="kser")
                nc.vector.tensor_scalar_add(kse_row, ks_ps, EPS)
                kb_ps = p15ps.tile([128, CQ], F32, tag="kbps")
                nc.tensor.matmul(kb_ps, lhsT=ones_row, rhs=kse_row)
                for r in range(GRP):
                    nc.vector.tensor_copy(kse_rep[:, r * CQ:(r + 1) * CQ],
                                          kb_ps)

                for g in range(NGRP):
                    prod = p15sm.tile([128, GRP * CQ], F32, tag="prod")
                    nc.vector.tensor_mul(
                        prod, qbuf[:, g * GRP * CQ:(g + 1) * GRP * CQ],
                        kse_rep)
                    dot = p15sm.tile([128, GRP], F32, tag="dot")
                    nc.vector.reduce_sum(
                        dot, prod.rearrange("p (c k) -> p c k", k=CQ),
                        axis=AX.X)
                    tg = p15sm.tile([128, GRP], F32, tag="tg")
                    nc.vector.tensor_scalar_add(tg, dot, float(N))
                    nc.vector.reciprocal(tg, tg)
                    nc.vector.tensor_scalar_mul(tg, tg, gamma_bc[:, 0:1])

                    for k in range(GRP):
                        sub = g * GRP + k
                        tr_in = p15sm.tile([128, CQ + 1], F32, tag="trin")
                        nc.vector.tensor_scalar_mul(
                            tr_in[:, 0:CQ],
                            qbuf[:, sub * CQ:(sub + 1) * CQ], tg[:, k:k + 1])
                        nc.scalar.copy(tr_in[:, CQ:CQ + 1], tg[:, k:k + 1])
                        tr_ps = trps.tile([CQ + 1, 128], F32, tag="trps")
                        nc.tensor.transpose(tr_ps, tr_in, ident)
                        nc.vector.tensor_copy(
                            qtaug[:, sub * SUB:(sub + 1) * SUB], tr_ps)

            # ---------------- phase 2 ----------------
            P2C = 1024
            with tc.tile_pool(name="p2ps", bufs=3, space="PSUM") as p2ps, \
                 tc.tile_pool(name="p2sb", bufs=2) as p2sb:
                lhs0 = mt_sb[:, 0:128]
                lhs1 = mt_sb[:, 128:C]
                for ci in range(N // P2C):
                    n0 = ci * P2C
                    ob0 = p2sb.tile([128, P2C], F32, tag="ob0")
                    ob1 = p2sb.tile([64, P2C], F32, tag="ob1")
                    for h in range(P2C // 512):
                        hs = slice(h * 512, (h + 1) * 512)
                        rhs = qtaug[:, n0 + h * 512:n0 + (h + 1) * 512]
                        o0 = p2ps.tile([128, 512], F32, tag="o0")
                        _mm(nc, o0, lhs0, rhs, FP32R_PHASE2)
                        nc.vector.tensor_copy(ob0[:, hs], o0)
                        o1 = p2ps.tile([64, 512], F32, tag="o1")
                        _mm(nc, o1, lhs1, rhs, FP32R_PHASE2)
                        nc.vector.tensor_copy(ob1[:, hs], o1)
                    nc.sync.dma_start(out=out[0:128, n0:n0 + P2C], in_=ob0)
                    nc.sync.dma_start(out=out[128:C, n0:n0 + P2C], in_=ob1)

    nc.compile()
    return nc


_NC = None


def _get_program():
    global _NC
    if _NC is None:
        _NC = build_program()
    return _NC


def _padf(a):
    out = np.zeros((a.shape[0], PADF), np.float32)
    out[:, :a.shape[1]] = a
    return out


def _host_prep(Wq, bq, Wk, bk, Wv, bv):
    WqkT = np.ascontiguousarray(np.concatenate([Wq, Wk], axis=0).T)  # [192, 64]
    bqk = np.concatenate([bq, bk], axis=0)[None, :]                  # [1, 64]
    wqk1 = _padf(WqkT[:128])
    wqk2 = _padf(np.concatenate([WqkT[128:], bqk], axis=0))
    WvT = np.ascontiguousarray(Wv.T)                                 # [192, 192]
    wv1 = _padf(WvT[:128])
    wv2 = _padf(np.concatenate([WvT[128:], bv[None, :]], axis=0))
    return wqk1, wqk2, wv1, wv2


def kernel(x, x1, Wq, bq, Wk, bk, Wv, bv, gamma):
    x = np.asarray(x, dtype=np.float32)
    x1 = np.asarray(x1, dtype=np.float32)
    wqk1, wqk2, wv1, wv2 = _host_prep(
        np.asarray(Wq, np.float32), np.asarray(bq, np.float32),
        np.asarray(Wk, np.float32), np.asarray(bk, np.float32),
        np.asarray(Wv, np.float32), np.asarray(bv, np.float32))
    gam = np.asarray(gamma, np.float32).reshape(1, 1)

    nc = _get_program()
    ones_one = np.ones((1, CHUNK), np.float32)
    in_maps = []
    for b in range(B):
        in_maps.append({
            "x1": np.ascontiguousarray(x1[b].reshape(C, N)),
            "x0": np.ascontiguousarray(x[b].reshape(C, N)),
            "wqk1": wqk1, "wqk2": wqk2, "wv1": wv1, "wv2": wv2,
            "gam": gam, "ones_d": ones_one,
        })
    res = run_bass_kernel_spmd(nc, in_maps, list(range(N_CORES)))
    outs = [res.results[b]["out"].reshape(C, H, W) for b in range(B)]
    return np.stack(outs, axis=0)


# revision 11
# speedup vs baseline: 1.1340x; 1.1013x over previous
"""Trainium2 Bass kernel for the sparse-attention nn.Module.

Math (per batch b, all fp32):
    Q = Wq @ x1 + bq            [32, N]     (N = 128*128 = 16384)
    K = Wk @ x1 + bk            [32, N]
    V = Wv @ x  + bv            [192, N]
    Qn = Q / ||Q||_col, Kn = K / ||K||_col          (norm over channel dim)
    ksum[m]   = sum_n Kn[m, n]
    tailor[n] = 1 / (N + sum_m Qn[m, n] * (ksum[m] + EPS))
    vsum[c]   = sum_n V[c, n]
    matrix[m, c] = sum_n Kn[m, n] V[c, n]
    out[c, n] = gamma * tailor[n] * (vsum[c] + sum_m Qn[m, n] matrix[m, c])

Distribution: data-parallel over batch. B == 8 == n_cores; each core gets one
batch slice, no collectives. Host reshapes/stacks.

Device algorithm (pos-major, one pass over N + one small pass):
  phase 1 per 128-position sub-chunk s (128 of them):
    qk_ps[128, 64]  = [x1 | 1s]^T-slice matmuls against [WqkT; bqk]  (bias via
                      ones-row fold), pos-major: partition = position.
    v_ps[128, 192]  = same for V.
    per-position norms of Q and K via ACT Square(accum) + Sqrt + DVE recip.
    Qn stored to qbuf[128, 32*128]; kn_aug[128, 33] = [Kn | 1]; vt[128, 193] =
    [V^T | 1].
    mt_ps[33, 193] += kn_aug^T @ vt   -- one accumulating matmul: rows 0..31 =
                      matrix[m, c], row 32 = vsum[c], col 192 = ksum[m].
  phase 1.5: ksum row extraction (tiny PE transpose), broadcast of
    (ksum + EPS) via Kc=1 matmul, tailor per position, fold gamma into tailor,
    build Qnt_aug[33, N] = transpose of [Qn * tailor_g | tailor_g] per chunk.
  phase 2 per 512-position chunk: out[c, n] = mt_sb[:, c]^T @ Qnt_aug[:, n]
    (Kc = 33), direct [c, n] layout, DMA out. Optionally fp32r (1 cyc/row at
    free-dim 512 vs 4 for fp32).
"""

import numpy as np

import concourse.bass as bass
import concourse.mybir as mybir
import concourse.tile as tile
from concourse import bacc
from concourse.bass_utils import run_bass_kernel_spmd
from concourse.masks import make_identity

F32 = mybir.dt.float32
AX = mybir.AxisListType
AF = mybir.ActivationFunctionType

N_CORES = 8
B, C, H, W = 8, 192, 128, 128
CQ = 32
N = H * W          # 16384
EPS = 1e-6

CHUNK = 512        # positions per DMA chunk
NCHUNK = N // CHUNK            # 32
SUB = 128          # positions per matmul sub-chunk
NSUB = N // SUB                # 128
SUBS_PER_CHUNK = CHUNK // SUB
GRP = 16           # sub-chunks per phase-1.5 group
NGRP = NSUB // GRP             # 8

# fp32r runs the PE at 1 cycle/row (vs 4 for fp32, which also needs 2 HW
# passes) when the matmul free dim is >= 256 -- so pad rhs free dims to 256.
FP32R_PHASE1 = True
FP32R_PHASE2 = True
PADF = 256
F32R = mybir.dt.float32r
DT1 = F32R if FP32R_PHASE1 else F32   # phase-1 matmul operand storage dtype
DT2 = F32R if FP32R_PHASE2 else F32   # phase-2 matmul operand storage dtype


def _mm(nc, out, lhsT, rhs, fp32r, **kw):
    nc.tensor.matmul(out, lhsT=lhsT, rhs=rhs, **kw)


def build_program():
    nc = bacc.Bacc("TRN2", target_bir_lowering=False, debug=False,
                   num_devices=N_CORES)

    x1 = nc.dram_tensor("x1", [C, N], DT1, kind="ExternalInput").ap()
    x0 = nc.dram_tensor("x0", [C, N], DT1, kind="ExternalInput").ap()
    wqk1 = nc.dram_tensor("wqk1", [128, PADF], DT1, kind="ExternalInput").ap()
    wqk2 = nc.dram_tensor("wqk2", [65, PADF], DT1, kind="ExternalInput").ap()
    wv1 = nc.dram_tensor("wv1", [128, PADF], DT1, kind="ExternalInput").ap()
    wv2 = nc.dram_tensor("wv2", [65, PADF], DT1, kind="ExternalInput").ap()
    gam = nc.dram_tensor("gam", [1, 1], F32, kind="ExternalInput").ap()
    ones_d = nc.dram_tensor("ones_d", [1, CHUNK], DT1, kind="ExternalInput").ap()
    out = nc.dram_tensor("out", [C, N], F32, kind="ExternalOutput").ap()

    with tile.TileContext(nc) as tc:
        with tc.tile_pool(name="singles", bufs=1) as singles, \
             tc.tile_pool(name="mtps", bufs=1, space="PSUM") as mtps_pool:
            w_qk1 = singles.tile([128, PADF], DT1)
            nc.sync.dma_start(out=w_qk1, in_=wqk1)
            w_qk2 = singles.tile([65, PADF], DT1)
            nc.sync.dma_start(out=w_qk2, in_=wqk2)
            w_v1 = singles.tile([128, PADF], DT1)
            nc.sync.dma_start(out=w_v1, in_=wv1)
            w_v2 = singles.tile([65, PADF], DT1)
            nc.sync.dma_start(out=w_v2, in_=wv2)
            gamma_bc = singles.tile([128, 1], F32)
            nc.sync.dma_start(out=gamma_bc, in_=gam.to_broadcast([128, 1]))
            ident = singles.tile([128, 128], F32)
            make_identity(nc, ident)
            ones_row = singles.tile([1, 128], F32)
            nc.vector.memset(ones_row, 1.0)

            # manually-rotated persistent tiles: the ones-regions are written
            # once here and never touched by the per-iteration writes, which
            # only cover the data region (avoids per-iteration memsets, which
            # cannot encode float32r anyway).
            x1b_t = [singles.tile([65, CHUNK], DT1, tag=f"x1b{i}", name=f"x1b{i}") for i in range(2)]
            xb_t = [singles.tile([65, CHUNK], DT1, tag=f"xb{i}", name=f"xb{i}") for i in range(2)]
            for t in x1b_t + xb_t:
                nc.sync.dma_start(out=t[64:65], in_=ones_d)
            kn_t = [singles.tile([128, CQ + 1], DT1, tag=f"kn{i}", name=f"kn{i}") for i in range(4)]
            for t in kn_t:
                nc.sync.dma_start(
                    out=t[:, CQ:CQ + 1],
                    in_=ones_d[0:1, 0:1].to_broadcast([128, 1]))
            vt_t = [singles.tile([128, PADF], DT1, tag=f"vt{i}", name=f"vt{i}") for i in range(4)]
            for t in vt_t:
                nc.sync.dma_start(
                    out=t[:, C:PADF],
                    in_=ones_d[0:1, 0:PADF - C].to_broadcast([128, PADF - C]))

            qbuf = singles.tile([128, NSUB * CQ], F32)     # Qn, pos-major
            qtaug = singles.tile([CQ + 1, N], DT2)         # Qnt_aug, chan-major
            mt_sb = singles.tile([CQ + 1, C + 1], DT2)     # matrix_aug
            kse_rep = singles.tile([128, GRP * CQ], F32)   # (ksum+EPS) bcast

            mt_ps = mtps_pool.tile([CQ + 1, PADF], F32)

            # ---------------- phase 1 ----------------
            with tc.tile_pool(name="xin", bufs=2) as xin, \
                 tc.tile_pool(name="qkps", bufs=3, space="PSUM") as qkps, \
                 tc.tile_pool(name="vps", bufs=3, space="PSUM") as vps, \
                 tc.tile_pool(name="p1sm", bufs=4) as p1sm:
                for ci in range(NCHUNK):
                    n0 = ci * CHUNK
                    x1a = xin.tile([128, CHUNK], DT1, tag="x1a")
                    nc.gpsimd.dma_start(out=x1a, in_=x1[0:128, n0:n0 + CHUNK])
                    x1b = x1b_t[ci % 2]
                    nc.gpsimd.dma_start(out=x1b[0:64], in_=x1[128:C, n0:n0 + CHUNK])
                    xa = xin.tile([128, CHUNK], DT1, tag="xa")
                    nc.gpsimd.dma_start(out=xa, in_=x0[0:128, n0:n0 + CHUNK])
                    xb = xb_t[ci % 2]
                    nc.gpsimd.dma_start(out=xb[0:64], in_=x0[128:C, n0:n0 + CHUNK])

                    for si in range(SUBS_PER_CHUNK):
                        sub = ci * SUBS_PER_CHUNK + si
                        sl = slice(si * SUB, (si + 1) * SUB)

                        qk_ps = qkps.tile([128, PADF], F32, tag="qk")
                        _mm(nc, qk_ps, x1a[:, sl], w_qk1, FP32R_PHASE1,
                            start=True, stop=False)
                        _mm(nc, qk_ps, x1b[:, sl], w_qk2, FP32R_PHASE1,
                            start=False, stop=True)
                        v_ps = vps.tile([128, PADF], F32, tag="v")
                        _mm(nc, v_ps, xa[:, sl], w_v1, FP32R_PHASE1,
                            start=True, stop=False)
                        _mm(nc, v_ps, xb[:, sl], w_v2, FP32R_PHASE1,
                            start=False, stop=True)

                        # per-position norms of Q and K (free-dim reduce)
                        scr = p1sm.tile([128, 2 * CQ], F32, tag="scr")
                        nc.scalar.activation(out=scr, in_=qk_ps[:, 0:2 * CQ],
                                             func=AF.Square)
                        sq2 = p1sm.tile([128, 2], F32, tag="sq2")
                        nc.vector.reduce_sum(
                            sq2, scr.rearrange("p (c k) -> p c k", k=CQ),
                            axis=AX.X)
                        rn2 = p1sm.tile([128, 2], F32, tag="rn2")
                        nc.scalar.sqrt(rn2, sq2)
                        nc.vector.reciprocal(rn2, rn2)

                        nc.vector.tensor_scalar_mul(
                            qbuf[:, sub * CQ:(sub + 1) * CQ],
                            qk_ps[:, 0:CQ], rn2[:, 0:1])
                        kn = kn_t[sub % 4]
                        nc.vector.tensor_scalar_mul(
                            kn[:, 0:CQ], qk_ps[:, CQ:2 * CQ], rn2[:, 1:2])
                        vt = vt_t[sub % 4]
                        nc.vector.tensor_copy(vt[:, 0:C], v_ps[:, 0:C])

                        _mm(nc, mt_ps, kn, vt, FP32R_PHASE1,
                            start=(sub == 0), stop=(sub == NSUB - 1))

            # ---------------- phase 1.5 ----------------
            nc.vector.tensor_copy(mt_sb, mt_ps[:, 0:C + 1])

            with tc.tile_pool(name="p15ps", bufs=2, space="PSUM") as p15ps, \
                 tc.tile_pool(name="p15sm", bufs=3) as p15sm, \
                 tc.tile_pool(name="trps", bufs=3, space="PSUM") as trps:
                # ksum (col 192 of mt_sb, rows 0..31) -> row, + EPS, bcast
                ks_ps = p15ps.tile([1, CQ], F32, tag="ksps")
                nc.tensor.transpose(ks_ps, mt_sb[0:CQ, C:C + 1].bitcast(F32),
                                    ident[0:CQ, 0:CQ])
                kse_row = p15sm.tile([1, CQ], F32, tag="kser")
                nc.vector.tensor_scalar_add(kse_row, ks_ps, EPS)
                kb_ps = p15ps.tile([128, CQ], F32, tag="kbps")
                nc.tensor.matmul(kb_ps, lhsT=ones_row, rhs=kse_row)
                for r in range(GRP):
                    nc.vector.tensor_copy(kse_rep[:, r * CQ:(r + 1) * CQ],
                                          kb_ps)

                for g in range(NGRP):
                    prod = p15sm.tile([128, GRP * CQ], F32, tag="prod")
                    nc.vector.tensor_mul(
                        prod, qbuf[:, g * GRP * CQ:(g + 1) * GRP * CQ],
                        kse_rep)
                    dot = p15sm.tile([128, GRP], F32, tag="dot")
                    nc.vector.reduce_sum(
                        dot, prod.rearrange("p (c k) -> p c k", k=CQ),
                        axis=AX.X)
                    tg = p15sm.tile([128, GRP], F32, tag="tg")
                    nc.vector.tensor_scalar_add(tg, dot, float(N))
                    nc.vector.reciprocal(tg, tg)
                    nc.vector.tensor_scalar_mul(tg, tg, gamma_bc[:, 0:1])

                    for k in range(GRP):
                        sub = g * GRP + k
                        tr_in = p15sm.tile([128, CQ + 1], F32, tag="trin")
                        nc.vector.tensor_scalar_mul(
                            tr_in[:, 0:CQ],
                            qbuf[:, sub * CQ:(sub + 1) * CQ], tg[:, k:k + 1])
                        nc.scalar.copy(tr_in[:, CQ:CQ + 1], tg[:, k:k + 1])
                        tr_ps = trps.tile([CQ + 1, 128], F32, tag="trps")
                        nc.tensor.transpose(tr_ps, tr_in, ident)
                        nc.vector.tensor_copy(
                            qtaug[:, sub * SUB:(sub + 1) * SUB], tr_ps)

            # ---------------- phase 2 ----------------
            P2C = 1024
            with tc.tile_pool(name="p2ps", bufs=3, space="PSUM") as p2ps, \
                 tc.tile_pool(name="p2sb", bufs=2) as p2sb:
                lhs0 = mt_sb[:, 0:128]
                lhs1 = mt_sb[:, 128:C]
                for ci in range(N // P2C):
                    n0 = ci * P2C
                    ob0 = p2sb.tile([128, P2C], F32, tag="ob0")
                    ob1 = p2sb.tile([64, P2C], F32, tag="ob1")
                    for h in range(P2C // 512):
                        hs = slice(h * 512, (h + 1) * 512)
                        rhs = qtaug[:, n0 + h * 512:n0 + (h + 1) * 512]
                        o0 = p2ps.tile([128, 512], F32, tag="o0")
                        _mm(nc, o0, lhs0, rhs, FP32R_PHASE2)
                        nc.vector.tensor_copy(ob0[:, hs], o0)
                        o1 = p2ps.tile([64, 512], F32, tag="o1")
                        _mm(nc, o1, lhs1, rhs, FP32R_PHASE2)
                        nc.vector.tensor_copy(ob1[:, hs], o1)
                    nc.sync.dma_start(out=out[0:128, n0:n0 + P2C], in_=ob0)
                    nc.sync.dma_start(out=out[128:C, n0:n0 + P2C], in_=ob1)

    nc.compile()
    return nc


_NC = None


def _get_program():
    global _NC
    if _NC is None:
        _NC = build_program()
    return _NC


def _padf(a):
    out = np.zeros((a.shape[0], PADF), np.float32)
    out[:, :a.shape[1]] = a
    return out


def _host_prep(Wq, bq, Wk, bk, Wv, bv):
    WqkT = np.ascontiguousarray(np.concatenate([Wq, Wk], axis=0).T)  # [192, 64]
    bqk = np.concatenate([bq, bk], axis=0)[None, :]                  # [1, 64]
    wqk1 = _padf(WqkT[:128])
    wqk2 = _padf(np.concatenate([WqkT[128:], bqk], axis=0))
    WvT = np.ascontiguousarray(Wv.T)                                 # [192, 192]
    wv1 = _padf(WvT[:128])
    wv2 = _padf(np.concatenate([WvT[128:], bv[None, :]], axis=0))
    return wqk1, wqk2, wv1, wv2


def kernel(x, x1, Wq, bq, Wk, bk, Wv, bv, gamma):
    x = np.asarray(x, dtype=np.float32)
    x1 = np.asarray(x1, dtype=np.float32)
    wqk1, wqk2, wv1, wv2 = _host_prep(
        np.asarray(Wq, np.float32), np.asarray(bq, np.float32),
        np.asarray(Wk, np.float32), np.asarray(bk, np.float32),
        np.asarray(Wv, np.float32), np.asarray(bv, np.float32))
    gam = np.asarray(gamma, np.float32).reshape(1, 1)

    nc = _get_program()
    ones_one = np.ones((1, CHUNK), np.float32)
    in_maps = []
    for b in range(B):
        in_maps.append({
            "x1": np.ascontiguousarray(x1[b].reshape(C, N)),
            "x0": np.ascontiguousarray(x[b].reshape(C, N)),
            "wqk1": wqk1, "wqk2": wqk2, "wv1": wv1, "wv2": wv2,
            "gam": gam, "ones_d": ones_one,
        })
    res = run_bass_kernel_spmd(nc, in_maps, list(range(N_CORES)))
    outs = [res.results[b]["out"].reshape(C, H, W) for b in range(B)]
    return np.stack(outs, axis=0)


# revision 13
# speedup vs baseline: 1.2215x; 1.0772x over previous
"""Trainium2 Bass kernel for the sparse-attention nn.Module.

Math (per batch b, all fp32):
    Q = Wq @ x1 + bq            [32, N]     (N = 128*128 = 16384)
    K = Wk @ x1 + bk            [32, N]
    V = Wv @ x  + bv            [192, N]
    Qn = Q / ||Q||_col, Kn = K / ||K||_col          (norm over channel dim)
    ksum[m]   = sum_n Kn[m, n]
    tailor[n] = 1 / (N + sum_m Qn[m, n] * (ksum[m] + EPS))
    vsum[c]   = sum_n V[c, n]
    matrix[m, c] = sum_n Kn[m, n] V[c, n]
    out[c, n] = gamma * tailor[n] * (vsum[c] + sum_m Qn[m, n] matrix[m, c])

Distribution: data-parallel over batch. B == 8 == n_cores; each core gets one
batch slice, no collectives. Host reshapes/stacks.

Device algorithm (pos-major, one pass over N + one small pass):
  phase 1 per 128-position sub-chunk s (128 of them):
    qk_ps[128, 64]  = [x1 | 1s]^T-slice matmuls against [WqkT; bqk]  (bias via
                      ones-row fold), pos-major: partition = position.
    v_ps[128, 192]  = same for V.
    per-position norms of Q and K via ACT Square(accum) + Sqrt + DVE recip.
    Qn stored to qbuf[128, 32*128]; kn_aug[128, 33] = [Kn | 1]; vt[128, 193] =
    [V^T | 1].
    mt_ps[33, 193] += kn_aug^T @ vt   -- one accumulating matmul: rows 0..31 =
                      matrix[m, c], row 32 = vsum[c], col 192 = ksum[m].
  phase 1.5: ksum row extraction (tiny PE transpose), broadcast of
    (ksum + EPS) via Kc=1 matmul, tailor per position, fold gamma into tailor,
    build Qnt_aug[33, N] = transpose of [Qn * tailor_g | tailor_g] per chunk.
  phase 2 per 512-position chunk: out[c, n] = mt_sb[:, c]^T @ Qnt_aug[:, n]
    (Kc = 33), direct [c, n] layout, DMA out. Optionally fp32r (1 cyc/row at
    free-dim 512 vs 4 for fp32).
"""

import numpy as np

import concourse.bass as bass
import concourse.mybir as mybir
import concourse.tile as tile
from concourse import bacc
from concourse.bass_utils import run_bass_kernel_spmd
from concourse.masks import make_identity

F32 = mybir.dt.float32
AX = mybir.AxisListType
AF = mybir.ActivationFunctionType

N_CORES = 8
B, C, H, W = 8, 192, 128, 128
CQ = 32
N = H * W          # 16384
EPS = 1e-6

CHUNK = 512        # positions per DMA chunk
NCHUNK = N // CHUNK            # 32
SUB = 128          # positions per matmul sub-chunk
NSUB = N // SUB                # 128
SUBS_PER_CHUNK = CHUNK // SUB
GRP = 16           # sub-chunks per phase-1.5 group
NGRP = NSUB // GRP             # 8

# fp32r runs the PE at 1 cycle/row (vs 4 for fp32, which also needs 2 HW
# passes) when the matmul free dim is >= 256 -- so pad rhs free dims to 256.
FP32R_PHASE1 = True
FP32R_PHASE2 = True
PADF = 256
F32R = mybir.dt.float32r
DT1 = F32R if FP32R_PHASE1 else F32   # phase-1 matmul operand storage dtype
DT2 = F32R if FP32R_PHASE2 else F32   # phase-2 matmul operand storage dtype


def _mm(nc, out, lhsT, rhs, fp32r, **kw):
    nc.tensor.matmul(out, lhsT=lhsT, rhs=rhs, **kw)


def build_program():
    nc = bacc.Bacc("TRN2", target_bir_lowering=False, debug=False,
                   num_devices=N_CORES)

    x1 = nc.dram_tensor("x1", [C, N], DT1, kind="ExternalInput").ap()
    x0 = nc.dram_tensor("x0", [C, N], DT1, kind="ExternalInput").ap()
    wqk1 = nc.dram_tensor("wqk1", [128, PADF], DT1, kind="ExternalInput").ap()
    wqk2 = nc.dram_tensor("wqk2", [65, PADF], DT1, kind="ExternalInput").ap()
    wv1 = nc.dram_tensor("wv1", [128, PADF], DT1, kind="ExternalInput").ap()
    wv2 = nc.dram_tensor("wv2", [65, PADF], DT1, kind="ExternalInput").ap()
    gam = nc.dram_tensor("gam", [1, 1], F32, kind="ExternalInput").ap()
    ones_d = nc.dram_tensor("ones_d", [1, CHUNK], DT1, kind="ExternalInput").ap()
    out = nc.dram_tensor("out", [C, N], F32, kind="ExternalOutput").ap()

    with tile.TileContext(nc) as tc:
        with tc.tile_pool(name="singles", bufs=1) as singles, \
             tc.tile_pool(name="mtps", bufs=1, space="PSUM") as mtps_pool:
            w_qk1 = singles.tile([128, PADF], DT1)
            nc.sync.dma_start(out=w_qk1, in_=wqk1)
            w_qk2 = singles.tile([65, PADF], DT1)
            nc.sync.dma_start(out=w_qk2, in_=wqk2)
            w_v1 = singles.tile([128, PADF], DT1)
            nc.sync.dma_start(out=w_v1, in_=wv1)
            w_v2 = singles.tile([65, PADF], DT1)
            nc.sync.dma_start(out=w_v2, in_=wv2)
            gamma_bc = singles.tile([128, 1], F32)
            nc.sync.dma_start(out=gamma_bc, in_=gam.to_broadcast([128, 1]))
            ident = singles.tile([128, 128], F32)
            make_identity(nc, ident)
            ones_row = singles.tile([1, 128], F32)
            nc.vector.memset(ones_row, 1.0)

            # manually-rotated persistent tiles: the ones-regions are written
            # once here and never touched by the per-iteration writes, which
            # only cover the data region (avoids per-iteration memsets, which
            # cannot encode float32r anyway).
            x1b_t = [singles.tile([65, CHUNK], DT1, tag=f"x1b{i}", name=f"x1b{i}") for i in range(2)]
            xb_t = [singles.tile([65, CHUNK], DT1, tag=f"xb{i}", name=f"xb{i}") for i in range(2)]
            for t in x1b_t + xb_t:
                nc.sync.dma_start(out=t[64:65], in_=ones_d)
            kn_t = [singles.tile([128, CQ + 1], DT1, tag=f"kn{i}", name=f"kn{i}") for i in range(4)]
            for t in kn_t:
                nc.sync.dma_start(
                    out=t[:, CQ:CQ + 1],
                    in_=ones_d[0:1, 0:1].to_broadcast([128, 1]))
            vt_t = [singles.tile([128, PADF], DT1, tag=f"vt{i}", name=f"vt{i}") for i in range(4)]
            for t in vt_t:
                nc.sync.dma_start(
                    out=t[:, C:PADF],
                    in_=ones_d[0:1, 0:PADF - C].to_broadcast([128, PADF - C]))

            qbuf = singles.tile([128, NSUB * CQ], F32)     # Qn, pos-major
            qtaug = singles.tile([CQ + 1, N], DT2)         # Qnt_aug, chan-major
            mt_sb = singles.tile([CQ + 1, C + 1], DT2)     # matrix_aug
            kse_sb = singles.tile([128, CQ], F32)          # (ksum+EPS) bcast

            mt_ps = mtps_pool.tile([CQ + 1, PADF], F32)

            # ---------------- phase 1 ----------------
            with tc.tile_pool(name="xin", bufs=2) as xin, \
                 tc.tile_pool(name="qkps", bufs=3, space="PSUM") as qkps, \
                 tc.tile_pool(name="vps", bufs=3, space="PSUM") as vps, \
                 tc.tile_pool(name="p1sm", bufs=4) as p1sm:
                for ci in range(NCHUNK):
                    n0 = ci * CHUNK
                    x1a = xin.tile([128, CHUNK], DT1, tag="x1a")
                    nc.gpsimd.dma_start(out=x1a, in_=x1[0:128, n0:n0 + CHUNK])
                    x1b = x1b_t[ci % 2]
                    nc.gpsimd.dma_start(out=x1b[0:64], in_=x1[128:C, n0:n0 + CHUNK])
                    xa = xin.tile([128, CHUNK], DT1, tag="xa")
                    nc.gpsimd.dma_start(out=xa, in_=x0[0:128, n0:n0 + CHUNK])
                    xb = xb_t[ci % 2]
                    nc.gpsimd.dma_start(out=xb[0:64], in_=x0[128:C, n0:n0 + CHUNK])

                    for si in range(SUBS_PER_CHUNK):
                        sub = ci * SUBS_PER_CHUNK + si
                        sl = slice(si * SUB, (si + 1) * SUB)

                        qk_ps = qkps.tile([128, PADF], F32, tag="qk")
                        _mm(nc, qk_ps, x1b[:, sl], w_qk2, FP32R_PHASE1,
                            start=True, stop=False)
                        _mm(nc, qk_ps, x1a[:, sl], w_qk1, FP32R_PHASE1,
                            start=False, stop=True)
                        v_ps = vps.tile([128, PADF], F32, tag="v")
                        _mm(nc, v_ps, xb[:, sl], w_v2, FP32R_PHASE1,
                            start=True, stop=False)
                        _mm(nc, v_ps, xa[:, sl], w_v1, FP32R_PHASE1,
                            start=False, stop=True)

                        # per-position norms of Q and K (free-dim reduce)
                        scr = p1sm.tile([128, 2 * CQ], F32, tag="scr")
                        nc.scalar.activation(out=scr, in_=qk_ps[:, 0:2 * CQ],
                                             func=AF.Square)
                        sq2 = p1sm.tile([128, 2], F32, tag="sq2")
                        nc.vector.reduce_sum(
                            sq2, scr.rearrange("p (c k) -> p c k", k=CQ),
                            axis=AX.X)
                        rn2 = p1sm.tile([128, 2], F32, tag="rn2")
                        nc.scalar.sqrt(rn2, sq2)
                        nc.vector.reciprocal(rn2, rn2)

                        nc.vector.tensor_scalar_mul(
                            qbuf[:, sub * CQ:(sub + 1) * CQ],
                            qk_ps[:, 0:CQ], rn2[:, 0:1])
                        kn = kn_t[sub % 4]
                        nc.vector.tensor_scalar_mul(
                            kn[:, 0:CQ], qk_ps[:, CQ:2 * CQ], rn2[:, 1:2])
                        vt = vt_t[sub % 4]
                        nc.vector.tensor_copy(vt[:, 0:C], v_ps[:, 0:C])

                        _mm(nc, mt_ps, kn, vt, FP32R_PHASE1,
                            start=(sub == 0), stop=(sub == NSUB - 1))

            # ---------------- phase 1.5 ----------------
            nc.vector.tensor_copy(mt_sb, mt_ps[:, 0:C + 1])

            with tc.tile_pool(name="p15ps", bufs=2, space="PSUM") as p15ps, \
                 tc.tile_pool(name="p15sm", bufs=3) as p15sm, \
                 tc.tile_pool(name="trps", bufs=3, space="PSUM") as trps:
                # ksum (col 192 of mt_sb, rows 0..31) -> row, + EPS, bcast
                ks_ps = p15ps.tile([1, CQ], F32, tag="ksps")
                nc.tensor.transpose(ks_ps, mt_sb[0:CQ, C:C + 1].bitcast(F32),
                                    ident[0:CQ, 0:CQ])
                kse_row = p15sm.tile([1, CQ], F32, tag="kser")
                nc.vector.tensor_scalar_add(kse_row, ks_ps, EPS)
                kb_ps = p15ps.tile([128, CQ], F32, tag="kbps")
                nc.tensor.matmul(kb_ps, lhsT=ones_row, rhs=kse_row)
                nc.vector.tensor_copy(kse_sb, kb_ps)
                # stride-0 view [128, GRP(bcast), CQ] of the [128, CQ] bcast
                kse_b = bass.AP(tensor=kse_sb.tensor, offset=kse_sb.offset,
                                ap=[kse_sb.ap[0], [0, GRP], kse_sb.ap[1]])

                for g in range(NGRP):
                    qb_g = qbuf[:, g * GRP * CQ:(g + 1) * GRP * CQ] \
                        .rearrange("p (c k) -> p c k", k=CQ)
                    prod = p15sm.tile([128, GRP, CQ], F32, tag="prod")
                    nc.vector.tensor_mul(prod, qb_g, kse_b)
                    dot = p15sm.tile([128, GRP], F32, tag="dot")
                    nc.vector.reduce_sum(dot, prod, axis=AX.X)
                    tg = p15sm.tile([128, GRP], F32, tag="tg")
                    nc.vector.tensor_scalar_add(tg, dot, float(N))
                    nc.vector.reciprocal(tg, tg)
                    nc.vector.tensor_scalar_mul(tg, tg, gamma_bc[:, 0:1])

                    # qs[:, k, 0:32] = Qn * tailor_g[k]; qs[:, k, 32] = tailor_g
                    qs = p15sm.tile([128, GRP, CQ + 1], F32, tag="qs")
                    tg_b = bass.AP(tensor=tg.tensor, offset=tg.offset,
                                   ap=[tg.ap[0], tg.ap[1], [0, CQ]])
                    nc.vector.tensor_mul(qs[:, :, 0:CQ], qb_g, tg_b)
                    nc.vector.tensor_copy(
                        qs[:, :, CQ:CQ + 1],
                        tg.rearrange("p (c u) -> p c u", u=1))

                    for k in range(GRP):
                        sub = g * GRP + k
                        tr_ps = trps.tile([CQ + 1, 128], F32, tag="trps")
                        nc.tensor.transpose(tr_ps, qs[:, k, :], ident)
                        nc.vector.tensor_copy(
                            qtaug[:, sub * SUB:(sub + 1) * SUB], tr_ps)

            # ---------------- phase 2 ----------------
            P2C = 1024
            with tc.tile_pool(name="p2ps", bufs=3, space="PSUM") as p2ps, \
                 tc.tile_pool(name="p2sb", bufs=2) as p2sb:
                lhs0 = mt_sb[:, 0:128]
                lhs1 = mt_sb[:, 128:C]
                for ci in range(N // P2C):
                    n0 = ci * P2C
                    ob0 = p2sb.tile([128, P2C], F32, tag="ob0")
                    ob1 = p2sb.tile([64, P2C], F32, tag="ob1")
                    for h in range(P2C // 512):
                        hs = slice(h * 512, (h + 1) * 512)
                        rhs = qtaug[:, n0 + h * 512:n0 + (h + 1) * 512]
                        o0 = p2ps.tile([128, 512], F32, tag="o0")
                        _mm(nc, o0, lhs0, rhs, FP32R_PHASE2)
                        nc.vector.tensor_copy(ob0[:, hs], o0)
                        o1 = p2ps.tile([64, 512], F32, tag="o1")
                        _mm(nc, o1, lhs1, rhs, FP32R_PHASE2)
                        nc.vector.tensor_copy(ob1[:, hs], o1)
                    nc.sync.dma_start(out=out[0:128, n0:n0 + P2C], in_=ob0)
                    nc.sync.dma_start(out=out[128:C, n0:n0 + P2C], in_=ob1)

    nc.compile()
    return nc


_NC = None


def _get_program():
    global _NC
    if _NC is None:
        _NC = build_program()
    return _NC


def _padf(a):
    out = np.zeros((a.shape[0], PADF), np.float32)
    out[:, :a.shape[1]] = a
    return out


def _host_prep(Wq, bq, Wk, bk, Wv, bv):
    WqkT = np.ascontiguousarray(np.concatenate([Wq, Wk], axis=0).T)  # [192, 64]
    bqk = np.concatenate([bq, bk], axis=0)[None, :]                  # [1, 64]
    wqk1 = _padf(WqkT[:128])
    wqk2 = _padf(np.concatenate([WqkT[128:], bqk], axis=0))
    WvT = np.ascontiguousarray(Wv.T)                                 # [192, 192]
    wv1 = _padf(WvT[:128])
    wv2 = _padf(np.concatenate([WvT[128:], bv[None, :]], axis=0))
    return wqk1, wqk2, wv1, wv2


def kernel(x, x1, Wq, bq, Wk, bk, Wv, bv, gamma):
    x = np.asarray(x, dtype=np.float32)
    x1 = np.asarray(x1, dtype=np.float32)
    wqk1, wqk2, wv1, wv2 = _host_prep(
        np.asarray(Wq, np.float32), np.asarray(bq, np.float32),
        np.asarray(Wk, np.float32), np.asarray(bk, np.float32),
        np.asarray(Wv, np.float32), np.asarray(bv, np.float32))
    gam = np.asarray(gamma, np.float32).reshape(1, 1)

    nc = _get_program()
    ones_one = np.ones((1, CHUNK), np.float32)
    in_maps = []
    for b in range(B):
        in_maps.append({
            "x1": np.ascontiguousarray(x1[b].reshape(C, N)),
            "x0": np.ascontiguousarray(x[b].reshape(C, N)),
            "wqk1": wqk1, "wqk2": wqk2, "wv1": wv1, "wv2": wv2,
            "gam": gam, "ones_d": ones_one,
        })
    res = run_bass_kernel_spmd(nc, in_maps, list(range(N_CORES)))
    outs = [res.results[b]["out"].reshape(C, H, W) for b in range(B)]
    return np.stack(outs, axis=0)
